# revision 28
# baseline (speedup 1.0000x reference)
"""Trainium2 Bass kernel for nn_Attention_77446850281941.

Computes, for dec_hidden [32,1024], enc_outputs [2048,32,1024], W [1,2048], b [1]:
    e[b,s]  = dec_hidden[b]@W[0,:1024] + enc_outputs[s,b,:]@W[0,1024:] + b[0]
    out     = softmax(tanh(e), axis=s)            -> [32, 2048] float32

Sharding: batch (32) is split across 8 NeuronCores (4 rows each); W/b are
replicated.  Softmax rows live entirely on one core, so no collectives.

The dominant cost is streaming enc (256 MB f32 over the chip).  Host-side
marshaling casts enc to fp16 (tolerance is 2e-2; fp16 + f32 PSUM
accumulation lands ~1e-3) and pre-transposes each core's shard so the
contraction axis e sits on SBUF partitions:

    enc_t[sb, p, h, c, s, b] = enc[sb*256 + h*128 + s, b, c*128 + p]

Per slab sb (2.1 MB, 16 KB/partition contiguous -> full DMA rate), the
TensorEngine does the whole weighted reduction as a matvec, consuming
128 elem/cycle (fully hidden under DMA):

    p_e[1, h, s, b] += w_cols[:, c].T @ slab[:, h, c, s, b]  (8 matmuls/h)

Everything downstream runs at half-slab (h) granularity so the epilogue
is only one half-slab deep: DVE adds the dec-bias row in PSUM, ScalarE
applies tanh in-place then exp into a partition-0 row buffer, DVE
accumulates per-b partial denominators, and a 2 KB SBUF->SBUF DMA
scatters each exp half-row to its 8 output partitions.  The first and
last slabs stream as two h-half DMAs to shorten ramp-in and drain.  The
epilogue combines partials, broadcasts reciprocals with a K=1 PE
matmul, multiplies, and stores 32 KB whose (s, b) decode happens in the
host-side unshard.
"""

import sys

import numpy as np

for _p in ("/opt/trn_rl_repo",):
    if _p not in sys.path:
        sys.path.insert(0, _p)

import concourse.bacc as bacc
import concourse.tile as tile
from concourse import mybir
from concourse.bass_utils import run_bass_kernel_spmd

F32 = mybir.dt.float32
F16 = mybir.dt.float16
SRC = 2048          # src_len
BATCH = 32
EH2 = 1024          # 2*enc_hid_dim
DH = 1024           # dec_hid_dim
NCORES = 8
BPC = BATCH // NCORES      # batch rows per core = 4
NCHUNK = EH2 // 128        # e-chunks = 8
SBLK = 256                 # s-values per slab
NSLAB = SRC // SBLK        # slabs per core = 8
SH = SBLK // 2             # s-values per PSUM-bank half = 128
SLAB_BUFS = NSLAB          # whole fp16 shard fits in SBUF; no recycling
OUTW = SRC * BPC // 128    # 64 output columns per partition

_NC_CACHE = {}


def build_nc():
    nc = bacc.Bacc("TRN2", target_bir_lowering=False, debug=False)

    enc = nc.dram_tensor("enc", [NSLAB, 128, 2, NCHUNK, SH, BPC], F16,
                         kind="ExternalInput").ap()
    wc = nc.dram_tensor("wc", [128, NCHUNK], F16, kind="ExternalInput").ap()
    out = nc.dram_tensor("out", [128, OUTW], F32, kind="ExternalOutput").ap()

    ADD = mybir.AluOpType.add
    MUL = mybir.AluOpType.mult
    ACT = mybir.ActivationFunctionType

    with tile.TileContext(nc) as tc:
        with (
            tc.tile_pool(name="consts", bufs=1) as consts,
            tc.tile_pool(name="slabs", bufs=SLAB_BUFS) as slabs,
            tc.tile_pool(name="small", bufs=1) as small,
            tc.tile_pool(name="psum", bufs=3, space="PSUM") as psum,
            tc.tile_pool(name="psum1", bufs=1, space="PSUM") as psum1,
        ):
            w_sb = consts.tile([128, NCHUNK], F16)
            nc.sync.dma_start(out=w_sb, in_=wc)
            ones128 = consts.tile([1, 128], F16)
            nc.gpsimd.memset(ones128, 1.0)

            # unnormalized exp rows (partition 0) and per-half partials
            exp_all = small.tile([1, NSLAB, 2, SH, BPC], F32)
            parts = small.tile([1, NSLAB, 2, BPC], F32)
            spread = small.tile([128, OUTW // BPC, BPC], F32)

            for sb in range(NSLAB):
                slab = slabs.tile([128, 2, NCHUNK, SH, BPC], F16)
                split = sb in (0, NSLAB - 1)
                if not split:
                    nc.sync.dma_start(out=slab, in_=enc[sb])
                p_e = psum.tile([1, 2, SH, BPC], F32)
                for h in range(2):
                    if split:
                        nc.sync.dma_start(
                            out=slab[:, h], in_=enc[sb][:, h])
                    # 8 chunk matvecs per PSUM-bank half (a matmul output
                    # cannot cross a 2 KB PSUM bank)
                    # the dec-bias is pre-folded into enc[:, :, e0] on the
                    # host, so the matvec yields e + bias directly
                    for c in range(NCHUNK):
                        nc.tensor.matmul(
                            p_e[:, h, :, :], w_sb[:, c:c + 1],
                            slab[:, h, c, :, :], start=(c == 0),
                            stop=(c == NCHUNK - 1))
                    nc.scalar.activation(out=p_e[:, h, :, :],
                                         in_=p_e[:, h, :, :], func=ACT.Tanh)
                    nc.scalar.activation(
                        out=exp_all[:, sb, h, :, :], in_=p_e[:, h, :, :],
                        func=ACT.Exp)
                    # per-b partial denominators for this half
                    for b_ in range(BPC):
                        nc.vector.tensor_reduce(
                            out=parts[:, sb, h, b_:b_ + 1],
                            in_=exp_all[:, sb, h, :, b_],
                            axis=mybir.AxisListType.X, op=ADD)
                    # scatter this half's exp row to its 8 output partitions
                    # (overlaps the stream; rides the scalar HWDGE ring)
                    nc.scalar.dma_start(
                        out=spread[sb * 16 + h * 8:sb * 16 + (h + 1) * 8],
                        in_=exp_all[:, sb, h, :, :])

            # denominators -> reciprocals -> broadcast to all partitions
            tot = small.tile([1, BPC], F32)
            for b_ in range(BPC):
                nc.vector.tensor_reduce(
                    out=tot[:, b_:b_ + 1], in_=parts[:, :, :, b_],
                    axis=mybir.AxisListType.XY, op=ADD)
            rec = small.tile([1, BPC], F16)
            with nc.allow_low_precision(reason="softmax recip bcast in fp16"):
                nc.vector.reciprocal(rec, tot)
            p_recb = psum1.tile([128, BPC], F32)
            nc.tensor.matmul(p_recb, ones128, rec)
            recb = small.tile([128, 1, BPC], F32)
            nc.scalar.activation(out=recb[:, 0, :], in_=p_recb,
                                 func=ACT.Identity)

            # normalize and store; (s, b) decode happens host-side
            out_sb = small.tile([128, OUTW // BPC, BPC], F32)
            nc.vector.tensor_tensor(
                out=out_sb, in0=spread,
                in1=recb.broadcast_to((128, OUTW // BPC, BPC)), op=MUL)
            nc.sync.dma_start(out=out, in_=out_sb)

    nc.finalize()
    return nc


def _get_nc():
    if "nc" not in _NC_CACHE:
        _NC_CACHE["nc"] = build_nc()
    return _NC_CACHE["nc"]


def make_in_maps(dec_hidden, enc_outputs, W, b):
    f32, f16 = np.float32, np.float16
    w_enc = np.asarray(W[0, DH:], dtype=f32)
    wc = np.ascontiguousarray(w_enc.reshape(NCHUNK, 128).T.astype(f16))
    w_dec = np.asarray(W[0, :DH], dtype=f32)
    bias = np.float32(b[0])
    dec_c = (np.asarray(dec_hidden, dtype=f32) @ w_dec + bias).astype(f32)
    enc_f = np.array(enc_outputs, dtype=f32)
    # fold the dec-bias into one enc element: with e0 = argmax|w_enc|,
    # enc[:, b, e0] += dec_c[b]/w_enc[e0] makes the matvec emit e + bias
    e0 = int(np.abs(w_enc).argmax())
    enc_f[:, :, e0] += (dec_c / w_enc[e0])[None, :]
    in_maps = []
    for i in range(NCORES):
        sl = slice(i * BPC, (i + 1) * BPC)
        # [2048, 4, 1024] -> [sb, h, s, b, c, p] -> [sb, p, h, c, s, b]
        enc_t = (enc_f[:, sl, :]
                 .reshape(NSLAB, 2, SH, BPC, NCHUNK, 128)
                 .transpose(0, 5, 1, 4, 2, 3)
                 .astype(f16))
        in_maps.append({
            "enc": np.ascontiguousarray(enc_t),
            "wc": wc,
        })
    return in_maps


def assemble_output(results):
    # out[m, j] = flat[m*64 + j]; flat order is (sb, h, s, b)
    outs = []
    for r in results:
        flat = r["out"].reshape(NSLAB, 2, SH, BPC)
        # -> [b, sb, h, s] -> [b, 2048]
        outs.append(flat.transpose(3, 0, 1, 2).reshape(BPC, SRC))
    return np.ascontiguousarray(np.concatenate(outs, axis=0)).astype(np.float32)


def kernel(dec_hidden, enc_outputs, W, b):
    nc = _get_nc()
    in_maps = make_in_maps(dec_hidden, enc_outputs, W, b)
    res = run_bass_kernel_spmd(nc, in_maps, core_ids=list(range(NCORES)))
    return assemble_output(res.results)
